# revision 12
# baseline (speedup 1.0000x reference)
"""DuoAttention kernel for 8 TRN2 NeuronCores (v2).

Math note: the reference's WINDOW == seq_len, so `local` and `full` are the
same MHA computation. The kernel computes one MHA pass; the duo gate reduces
to a per-batch scalar factor c[i] = (m[i] < 0.1) ? (1 - m[i]) : 1.0 applied
in the broadcast combine out[i, j] = c[i] * mha[j] (shape [B, B, S, D]).

Sharding: data-parallel over batch (2) x tensor-parallel over head groups
(4 groups x 4 heads). Each core computes QKV projections for its 256
features, attention for its 4 heads, and a partial output projection
(contribution of its 256 o-features to all 1024 output dims). The host sums
the 4 partials per batch, adds the (bv-folded) output bias, and applies the
gate.

v2 changes vs the previous kernel:
  - AV matmuls are column-tiled: the two heads of a pair run concurrently
    on array column groups (0,0)/(0,64), each 64-wide, doubling AV
    throughput. The softmax rowsum no longer rides the AV as a 65th output
    column; instead exp tiles are chain-summed on the DVE (bf16, 2x mode)
    and collapsed across partitions by a tiny col-tiled ones-matmul whose
    [128,64] stationary also broadcasts the result to all partitions --
    killing the old per-section gpsimd broadcast + 1-partition reciprocals.
  - k-projection bias dropped (exact: softmax is invariant to per-query
    shifts), v-projection bias dropped (exact: folded into the host-side
    output bias as bv @ Wo^T).
  - Inputs are host-pre-tiled so each activation tranche is one contiguous
    1 MB DMA with 8 KB packets (the old layout trickled 1 KB packets through
    ~76 dispatches at ~700 ns each, starving the head of the kernel).
  - Longer PE warmup spans the DMA head so HAM reaches 2.4 GHz before the
    scores stream starts.
"""

import sys

import numpy as np
import ml_dtypes

_REPO = "/opt/trn_rl_repo"
if _REPO not in sys.path:
    sys.path.insert(0, _REPO)

import concourse.bass as bass
import concourse.bacc as bacc
import concourse.mybir as mybir
import concourse.tile as tile
from concourse.bass_utils import run_bass_kernel_spmd

B, S, D, H = 2, 2048, 1024, 16
NCORES = 8
GROUPS = 4            # head groups (tensor parallel)
HPG = H // GROUPS     # 4 heads per group
DH = D // H           # 64
GF = HPG * DH         # 256 features per group
DC = D // 128         # 8 contraction chunks of 128
QT = S // 512         # 4 tranches of 512
KT = S // 128         # 16 key tiles of 128 per section

BF16 = mybir.dt.bfloat16
F32 = mybir.dt.float32

LAG = 20   # trail (attn@v / normalize) lag behind the scores/exp stream
RLAG = 2   # rowsum chain-add lag behind the exp stream
NWARM = 16


def build_nc():
    nc = bacc.Bacc("TRN2", target_bir_lowering=False, debug=False,
                   num_devices=NCORES)

    # activations host-tiled [tranche, partition(=D-chunk row), dc, 512]
    kT = nc.dram_tensor("kT", [QT, 128, DC, 512], BF16, kind="ExternalInput").ap()
    qT = nc.dram_tensor("qT", [QT, 128, DC, 512], BF16, kind="ExternalInput").ap()
    vT = nc.dram_tensor("vT", [QT, 128, DC, 512], BF16, kind="ExternalInput").ap()
    wqT = nc.dram_tensor("wqT", [2, 128, DC, 128], BF16, kind="ExternalInput").ap()
    wkT = nc.dram_tensor("wkT", [2, 128, DC, 128], BF16, kind="ExternalInput").ap()
    wvT = nc.dram_tensor("wvT", [128, DC, GF], BF16, kind="ExternalInput").ap()
    woT = nc.dram_tensor("woT", [128, 2, D], BF16, kind="ExternalInput").ap()
    bq = nc.dram_tensor("bq", [GF], F32, kind="ExternalInput").ap()
    out = nc.dram_tensor("out_part", [S, D], BF16, kind="ExternalOutput").ap()

    with tile.TileContext(nc) as tc:
        with (
            tc.tile_pool(name="const", bufs=1) as const,
            tc.tile_pool(name="acts", bufs=1) as acts,
            tc.tile_pool(name="sc", bufs=2, space="PSUM") as scp,
            tc.tile_pool(name="av", bufs=2, space="PSUM") as avp,
            tc.tile_pool(name="misc", bufs=2, space="PSUM") as miscp,
            tc.tile_pool(name="exp", bufs=LAG + 2) as exps,
            tc.tile_pool(name="racc", bufs=3) as raccp,
            tc.tile_pool(name="bri", bufs=2) as brip,
            tc.tile_pool(name="ot", bufs=1) as otp,
            tc.tile_pool(name="small", bufs=2) as small,
            tc.tile_pool(name="outs", bufs=3) as outsp,
        ):
            # ---- constants (memsets run during the boilerplate head)
            ones128 = const.tile([128, 128], BF16, tag="ones128")
            nc.vector.memset(ones128, 1.0)
            ones64 = ones128[:, 0:64]
            warm_rhs = const.tile([128, 512], BF16, tag="warm_rhs")
            nc.vector.memset(warm_rhs, 0.0)

            # ---- input DMAs, in first-needed order; one dispatch each.
            # q/k weights are split by head-pair so the scores-critical head
            # only waits on 2.5 MB instead of 3.5 MB.
            wk_sb = [const.tile([128, DC, 128], BF16, tag=f"wk{p}", name=f"wk{p}")
                     for p in range(2)]
            wq_sb = [const.tile([128, DC, 128], BF16, tag=f"wq{p}", name=f"wq{p}")
                     for p in range(2)]
            k_sb = [acts.tile([128, DC, 512], BF16, tag=f"k{t}", name=f"k{t}")
                    for t in range(QT)]
            q_sb = [acts.tile([128, DC, 512], BF16, tag=f"q{t}", name=f"q{t}")
                    for t in range(QT)]
            v_sb = [acts.tile([128, DC, 512], BF16, tag=f"v{t}", name=f"v{t}")
                    for t in range(QT)]
            nc.sync.dma_start(out=wk_sb[0], in_=wkT[0, :, :, :])
            nc.sync.dma_start(out=k_sb[0], in_=kT[0, :, :, :])
            nc.sync.dma_start(out=wq_sb[0], in_=wqT[0, :, :, :])
            nc.sync.dma_start(out=q_sb[0], in_=qT[0, :, :, :])
            bq_sb = const.tile([128, 2], F32, tag="bq")
            nc.sync.dma_start(out=bq_sb, in_=bq.rearrange("(t p) -> p t", p=128))
            nc.sync.dma_start(out=k_sb[1], in_=kT[1, :, :, :])
            nc.sync.dma_start(out=k_sb[2], in_=kT[2, :, :, :])
            nc.sync.dma_start(out=k_sb[3], in_=kT[3, :, :, :])
            wv_sb = const.tile([128, DC, GF], BF16, tag="wv")
            nc.sync.dma_start(out=wv_sb, in_=wvT)
            nc.sync.dma_start(out=v_sb[0], in_=vT[0, :, :, :])
            nc.sync.dma_start(out=q_sb[1], in_=qT[1, :, :, :])
            nc.sync.dma_start(out=v_sb[1], in_=vT[1, :, :, :])
            nc.sync.dma_start(out=q_sb[2], in_=qT[2, :, :, :])
            nc.sync.dma_start(out=v_sb[2], in_=vT[2, :, :, :])
            nc.sync.dma_start(out=v_sb[3], in_=vT[3, :, :, :])
            nc.sync.dma_start(out=q_sb[3], in_=qT[3, :, :, :])
            nc.sync.dma_start(out=wk_sb[1], in_=wkT[1, :, :, :])
            nc.sync.dma_start(out=wq_sb[1], in_=wqT[1, :, :, :])
            wo_sb = const.tile([128, 2, D], BF16, tag="wo")
            nc.sync.dma_start(out=wo_sb, in_=woT)

            # ---- persistent projection outputs
            kp_sl = [[acts.tile([128, 512], BF16, tag=f"kp{p}_{t}",
                                name=f"kp{p}_{t}") for t in range(QT)]
                     for p in range(2)]
            qp_sl = [[acts.tile([128, 512], BF16, tag=f"qp{p}_{t}",
                                name=f"qp{p}_{t}") for t in range(QT)]
                     for p in range(2)]
            vp_t = [acts.tile([128, GF], BF16, tag=f"vp{st}", name=f"vp{st}")
                    for st in range(KT)]
            oT_tiles = [otp.tile([128, 2, 512], BF16, tag=f"ot{qt}",
                                 name=f"oT{qt}") for qt in range(QT)]

            # ---- PE warmup: spans the DMA head so HAM reaches 2.4 GHz
            # before the scores stream starts (dependency-free)
            warm_ps = miscp.tile([128, 512], F32, tag="misc", name="warm_ps")
            for _ in range(NWARM):
                nc.tensor.matmul(warm_ps, ones128, warm_rhs,
                                 start=True, stop=True)
            # WAR-ordered read so the psum slot releases right after warmup
            nc.vector.tensor_copy(warm_rhs, warm_ps)

            # ---- q/k projections, drip-fed in halves (4 dc chunks each).
            # A half-open group holds one misc ring slot; between its two
            # halves at most one other alloc may occur (safe: its last read
            # is always emitted before any later alloc's first write).
            _fs_state = {}

            def proj_fs_half(w_sb, b_sb, dst_sl, x_sb, pair, t, half):
                key = (id(dst_sl), pair, t)
                if half == 0:
                    _fs_state[key] = miscp.tile([128, 512], F32, tag="misc",
                                                name="ps_fs")
                ps = _fs_state[key]
                for dc in range(4 * half, 4 * half + 4):
                    nc.tensor.matmul(
                        ps,
                        w_sb[pair][:, dc, :],
                        x_sb[t][:, dc, :],
                        start=(dc == 0), stop=(dc == DC - 1),
                    )
                if half == 1:
                    del _fs_state[key]
                    if b_sb is None:
                        nc.vector.tensor_copy(dst_sl[pair][t], ps)
                    else:
                        nc.vector.tensor_scalar_add(
                            dst_sl[pair][t], ps, b_sb[:, pair:pair + 1])

            def proj_fs_group(w_sb, b_sb, dst_sl, x_sb, pair, t):
                proj_fs_half(w_sb, b_sb, dst_sl, x_sb, pair, t, 0)
                proj_fs_half(w_sb, b_sb, dst_sl, x_sb, pair, t, 1)

            def proj_fs_quarter(w_sb, b_sb, dst_sl, x_sb, pair, t, qq):
                key = (id(dst_sl), pair, t)
                if qq == 0:
                    _fs_state[key] = miscp.tile([128, 512], F32, tag="misc",
                                                name="ps_fs")
                ps = _fs_state[key]
                for dc in range(2 * qq, 2 * qq + 2):
                    nc.tensor.matmul(
                        ps,
                        w_sb[pair][:, dc, :],
                        x_sb[t][:, dc, :],
                        start=(dc == 0), stop=(dc == DC - 1),
                    )
                if qq == 3:
                    del _fs_state[key]
                    if b_sb is None:
                        nc.vector.tensor_copy(dst_sl[pair][t], ps)
                    else:
                        nc.vector.tensor_scalar_add(
                            dst_sl[pair][t], ps, b_sb[:, pair:pair + 1])

            def proj_v_group(st):
                ps = miscp.tile([128, GF], F32, tag="misc", name="ps_v")
                for dc in range(DC):
                    nc.tensor.matmul(
                        ps,
                        v_sb[st // 4][:, dc, 128 * (st % 4):128 * (st % 4) + 128],
                        wv_sb[:, dc, :],
                        start=(dc == 0), stop=(dc == DC - 1),
                    )
                nc.vector.tensor_copy(vp_t[st], ps)

            # prologue projections for stream position 0
            proj_fs_group(wk_sb, None, kp_sl, k_sb, 0, 0)
            proj_fs_group(wq_sb, bq_sb, qp_sl, q_sb, 0, 0)


            # drip-fed projection jobs at fixed stream positions.
            # needs: kp[0][t] by slot 4t; kp[1][t] by slot 16+4t;
            # qp[0][qt] by slot 32qt; qp[1][qt] by slot 32qt+16.
            scheduled = {}

            def sched(pos, job):
                scheduled.setdefault(pos, []).append(job)

            def sched_halves(pos, w, pair, t):
                wsb, bsb, dst, xs = ((wq_sb, bq_sb, qp_sl, q_sb) if w == "q"
                                     else (wk_sb, None, kp_sl, k_sb))
                for h in range(2):
                    sched(pos + h,
                          (lambda hh: lambda: proj_fs_half(
                              wsb, bsb, dst, xs, pair, t, hh))(h))

            def sched_quarters(pos, w, pair, t):
                wsb, bsb, dst, xs = ((wq_sb, bq_sb, qp_sl, q_sb) if w == "q"
                                     else (wk_sb, None, kp_sl, k_sb))
                for qq in range(4):
                    sched(pos + qq + (1 if pos + qq >= 67 and pos <= 67 else 0),
                          (lambda q2: lambda: proj_fs_quarter(
                              wsb, bsb, dst, xs, pair, t, q2))(qq))

            # v-projections dripped just ahead of their trail consumer
            # (kt consumed at slot LAG+kt)
            for _kt in range(8):
                sched(13 + _kt, (lambda k2: lambda: proj_v_group(k2))(_kt))
            for _kt in range(8, 16):
                sched(28 + (_kt - 8), (lambda k2: lambda: proj_v_group(k2))(_kt))

            sched_halves(2, "k", 0, 1)
            sched_halves(5, "k", 0, 2)
            sched_halves(9, "k", 0, 3)
            sched_halves(13, "q", 0, 1)
            sched_quarters(24, "q", 0, 2)
            sched_quarters(44, "q", 0, 3)
            sched_quarters(48, "q", 1, 0)
            sched_quarters(52, "k", 1, 0)
            sched_quarters(56, "k", 1, 1)
            sched_quarters(60, "k", 1, 2)
            sched_quarters(64, "k", 1, 3)
            sched_quarters(72, "q", 1, 1)
            sched_quarters(84, "q", 1, 2)
            sched_quarters(92, "q", 1, 3)

            # ---- output projection (per (qt, sj): 512 outputs in 2 halves)
            _op_state = {}

            def outproj_half(qt, sj, do):
                if do == 0:
                    _op_state[(qt, sj)] = outsp.tile([128, D], BF16, tag="os",
                                                     name="outt")
                outt = _op_state[(qt, sj)]
                oT_t = oT_tiles[qt]
                ps = miscp.tile([128, 512], F32, tag="misc", name="ps_o")
                for fc in range(2):
                    nc.tensor.matmul(
                        ps,
                        oT_t[:, fc, 128 * sj:128 * sj + 128],
                        wo_sb[:, fc, 512 * do:512 * do + 512],
                        start=(fc == 0), stop=(fc == 1),
                    )
                nc.vector.tensor_copy(outt[:, 512 * do:512 * do + 512], ps)
                if do == 1:
                    del _op_state[(qt, sj)]
                    row = 512 * qt + 128 * sj
                    nc.sync.dma_start(out=out[row:row + 128, :], in_=outt)

            # ---- per-section state
            # pair-major: sections 0-3 = head-pair 0 over qt 0-3, 4-7 = pair 1.
            # This defers the kp[1]/qp[1] projections to mid-stream slots,
            # easing the early-phase PE congestion.
            sections = [(qt, p) for p in range(2) for qt in range(QT)]
            stream = [(sec, kt) for sec in range(8) for kt in range(KT)]

            av_ps = {}           # sec -> av psum tile [128, 512]
            ex_store = {}        # stream idx -> exp tile
            racc_state = {}      # (sec, parity) -> (last_ex_or_None, acc_or_None)
            racc_final = {}      # sec -> accF tile
            jobs = []            # outproj job queue

            def radd(j):
                """DVE rowsum chain-add for the exp tile of stream slot j."""
                s, kt = stream[j]
                par = kt % 2
                ex = ex_store[j]
                key = (s, par)
                if kt < 2:
                    racc_state[key] = (ex, None)
                    return
                prev_ex, acc = racc_state[key]
                nacc = raccp.tile([128, 1024], BF16, tag="racc", name="racc")
                if acc is None:
                    nc.vector.tensor_add(nacc, prev_ex, ex)
                else:
                    nc.vector.tensor_add(nacc, acc, ex)
                racc_state[key] = (None, nacc)
                if kt == 15:
                    _, acc_e = racc_state.pop((s, 0))
                    _, acc_o = racc_state.pop((s, 1))
                    accf = raccp.tile([128, 1024], BF16, tag="racc",
                                      name="raccf")
                    nc.vector.tensor_add(accf, acc_e, acc_o)
                    racc_final[s] = accf

            def normalize(s):
                qt, p = sections[s]
                av = av_ps.pop(s)
                accf = racc_final.pop(s)
                # collapse rowsum partials across partitions; the [128, 64]
                # all-ones stationary replicates the per-query rowsum onto 64
                # partitions per head (col-tiled pair, concurrent)
                rbt = miscp.tile([128, 512], F32, tag="misc", name="rbt")
                nc.tensor.matmul(rbt[0:64, :], ones64, accf[:, 0:512],
                                 start=True, stop=True)
                nc.tensor.matmul(rbt[64:128, :], ones64, accf[:, 512:1024],
                                 start=True, stop=True)
                bri = brip.tile([128, 512], F32, tag="bri", name="bri")
                nc.vector.reciprocal_approx_fast(bri, rbt)
                nc.vector.tensor_mul(oT_tiles[qt][:, p, :], av, bri)
                if p == 1:
                    jobs.extend((qt, sj, do)
                                for sj in range(4) for do in range(2))

            def trail(jdx):
                s, kt = stream[jdx]
                qt, p = sections[s]
                if kt == 0:
                    av_ps[s] = avp.tile([128, 512], F32, tag="av", name="av")
                av = av_ps[s]
                exa = ex_store.pop(jdx)
                # col-tiled head pair: h(2p) on psum partitions 0-63
                # (array cols 0-63), h(2p+1) on 64-127 -- concurrent
                nc.tensor.matmul(
                    av[0:64, :],
                    vp_t[kt][:, 128 * p:128 * p + 64],
                    exa[:, 0:512],
                    start=(kt == 0), stop=(kt == KT - 1),
                )
                nc.tensor.matmul(
                    av[64:128, :],
                    vp_t[kt][:, 128 * p + 64:128 * p + 128],
                    exa[:, 512:1024],
                    start=(kt == 0), stop=(kt == KT - 1),
                )
                if kt == KT - 1:
                    normalize(s)

            # ---- the stream
            tcur = 0
            rcur = 0
            for i in range(len(stream) + LAG):
                if i < len(stream):
                    s, kt = stream[i]
                    qt, p = sections[s]
                    sc = scp.tile([128, 1024], F32, tag="sc", name="sc")
                    # row-packed head pair: 2p on array rows 0-63, 2p+1 on
                    # rows 64-127 (sequential; scores are psum-port-bound)
                    nc.tensor.matmul(
                        sc[:, 0:512],
                        kp_sl[p][kt // 4][0:64, 128 * (kt % 4):128 * (kt % 4) + 128],
                        qp_sl[p][qt][0:64, :],
                        start=True, stop=True,
                    )
                    nc.tensor.matmul(
                        sc[:, 512:1024],
                        kp_sl[p][kt // 4][64:128, 128 * (kt % 4):128 * (kt % 4) + 128],
                        qp_sl[p][qt][64:128, :],
                        start=True, stop=True,
                    )
                    ex = exps.tile([128, 1024], BF16, tag="exp", name="ex")
                    nc.scalar.activation(
                        out=ex, in_=sc,
                        func=mybir.ActivationFunctionType.Exp,
                        scale=1.0 / np.sqrt(DH),
                    )
                    ex_store[i] = ex
                    for fn in scheduled.pop(i, []):
                        fn()
                    if i not in scheduled and not _fs_state and jobs:
                        _q, _sj, _do = jobs.pop(0)
                        outproj_half(_q, _sj, _do)
                if i >= RLAG and rcur <= i - RLAG and rcur < len(stream):
                    radd(rcur)
                    rcur += 1
                if i >= LAG and tcur < len(stream):
                    trail(tcur)
                    tcur += 1
                # gently accelerate the trail (one extra AV every other
                # slot, fits the per-slot PE slack) so the tail is short
                if i >= 68 and i % 2 == 0 and tcur < len(stream) and tcur <= i - 4:
                    trail(tcur)
                    tcur += 1
            while rcur < len(stream):
                radd(rcur)
                rcur += 1
            while tcur < len(stream):
                trail(tcur)
                tcur += 1
            while jobs:
                _q, _sj, _do = jobs.pop(0)
                outproj_half(_q, _sj, _do)

    nc.compile()
    return nc


_CACHE = {}


def _get_nc():
    if "nc" not in _CACHE:
        _CACHE["nc"] = build_nc()
    return _CACHE["nc"]


def _tile_acts(x):
    # [S, D] -> transpose -> [(c p), (t s)] -> [t, p, c, s] contiguous
    bf16 = ml_dtypes.bfloat16
    xt = x.T.reshape(DC, 128, QT, 512).transpose(2, 1, 0, 3)
    return np.ascontiguousarray(xt).astype(bf16)


def _tile_w(w):
    # [D, GF] -> [(c p), f] -> [p, c, f] contiguous
    bf16 = ml_dtypes.bfloat16
    return np.ascontiguousarray(
        w.reshape(DC, 128, GF).transpose(1, 0, 2)).astype(bf16)


def _tile_w_pair(w):
    # [D, GF] -> [pair, p, c, 128] contiguous
    bf16 = ml_dtypes.bfloat16
    return np.ascontiguousarray(
        w.reshape(DC, 128, 2, 128).transpose(2, 1, 0, 3)).astype(bf16)


def _prep_inputs(query, key, value, in_proj_w, in_proj_b, out_proj_w):
    bf16 = ml_dtypes.bfloat16
    wq, wk, wv = (in_proj_w[0:D], in_proj_w[D:2 * D], in_proj_w[2 * D:3 * D])
    bq = in_proj_b[0:D]

    qT = [_tile_acts(query[b]) for b in range(B)]
    kT = [_tile_acts(key[b]) for b in range(B)]
    vT = [_tile_acts(value[b]) for b in range(B)]

    in_maps = []
    for b in range(B):
        for g in range(GROUPS):
            fs = slice(GF * g, GF * (g + 1))
            woT = out_proj_w[:, fs].T   # [GF, D]
            in_maps.append({
                "qT": qT[b], "kT": kT[b], "vT": vT[b],
                "wqT": _tile_w_pair(np.ascontiguousarray(wq[fs].T)),
                "wkT": _tile_w_pair(np.ascontiguousarray(wk[fs].T)),
                "wvT": _tile_w(np.ascontiguousarray(wv[fs].T)),
                "woT": np.ascontiguousarray(
                    woT.reshape(2, 128, D).transpose(1, 0, 2)).astype(bf16),
                "bq": np.ascontiguousarray(bq[fs]).astype(np.float32),
            })
    return in_maps


def kernel(query, key, value, in_proj_w, in_proj_b, out_proj_w, out_proj_b,
           mask_w, mask_b, _run_kwargs=None):
    query = np.asarray(query, np.float32)
    key = np.asarray(key, np.float32)
    value = np.asarray(value, np.float32)
    in_proj_w = np.asarray(in_proj_w, np.float32)
    in_proj_b = np.asarray(in_proj_b, np.float32)
    out_proj_w = np.asarray(out_proj_w, np.float32)
    out_proj_b = np.asarray(out_proj_b, np.float32)
    mask_w = np.asarray(mask_w, np.float32)
    mask_b = np.asarray(mask_b, np.float32)

    in_maps = _prep_inputs(query, key, value, in_proj_w, in_proj_b, out_proj_w)
    nc = _get_nc()
    for _attempt in range(3):
        res = run_bass_kernel_spmd(nc, in_maps, core_ids=list(range(NCORES)),
                                   **(_run_kwargs or {}))
        parts = [np.asarray(r["out_part"], np.float32) for r in res.results]
        # guard against rare transient device glitches: partial outputs are
        # normally bounded well under 1
        if all(np.isfinite(p).all() and np.abs(p).max() < 100.0 for p in parts):
            break
    # the v-projection bias is exact to fold into the output bias:
    # o = attn @ (v + bv) -> attn@v + bv, so out += bv @ Wo^T
    bv = in_proj_b[2 * D:3 * D]
    eff_bias = out_proj_b + bv @ out_proj_w.T
    mha = np.stack(
        [sum(parts[b * GROUPS + g] for g in range(GROUPS)) for b in range(B)],
        axis=0,
    ) + eff_bias[None, None, :].astype(np.float32)

    logit = (query[:, -1] @ mask_w.T + mask_b).astype(np.float64)
    m = (1.0 / (1.0 + np.exp(-logit))).astype(np.float32).reshape(B)
    c = np.where(m < 0.1, np.float32(1.0) - m, np.float32(1.0))

    out_full = c[:, None, None, None] * mha[None, :, :, :]
    if _run_kwargs is not None:
        _CACHE["last_results"] = res
    return out_full.astype(np.float32)


# revision 13
# speedup vs baseline: 1.0146x; 1.0146x over previous
"""DuoAttention kernel for 8 TRN2 NeuronCores (v2).

Math note: the reference's WINDOW == seq_len, so `local` and `full` are the
same MHA computation. The kernel computes one MHA pass; the duo gate reduces
to a per-batch scalar factor c[i] = (m[i] < 0.1) ? (1 - m[i]) : 1.0 applied
in the broadcast combine out[i, j] = c[i] * mha[j] (shape [B, B, S, D]).

Sharding: data-parallel over batch (2) x tensor-parallel over head groups
(4 groups x 4 heads). Each core computes QKV projections for its 256
features, attention for its 4 heads, and a partial output projection
(contribution of its 256 o-features to all 1024 output dims). The host sums
the 4 partials per batch, adds the (bv-folded) output bias, and applies the
gate.

v2 changes vs the previous kernel:
  - AV matmuls are column-tiled: the two heads of a pair run concurrently
    on array column groups (0,0)/(0,64), each 64-wide, doubling AV
    throughput. The softmax rowsum no longer rides the AV as a 65th output
    column; instead exp tiles are chain-summed on the DVE (bf16, 2x mode)
    and collapsed across partitions by a tiny col-tiled ones-matmul whose
    [128,64] stationary also broadcasts the result to all partitions --
    killing the old per-section gpsimd broadcast + 1-partition reciprocals.
  - k-projection bias dropped (exact: softmax is invariant to per-query
    shifts), v-projection bias dropped (exact: folded into the host-side
    output bias as bv @ Wo^T).
  - Inputs are host-pre-tiled so each activation tranche is one contiguous
    1 MB DMA with 8 KB packets (the old layout trickled 1 KB packets through
    ~76 dispatches at ~700 ns each, starving the head of the kernel).
  - Longer PE warmup spans the DMA head so HAM reaches 2.4 GHz before the
    scores stream starts.
"""

import sys

import numpy as np
import ml_dtypes

_REPO = "/opt/trn_rl_repo"
if _REPO not in sys.path:
    sys.path.insert(0, _REPO)

import concourse.bass as bass
import concourse.bacc as bacc
import concourse.mybir as mybir
import concourse.tile as tile
from concourse.bass_utils import run_bass_kernel_spmd

B, S, D, H = 2, 2048, 1024, 16
NCORES = 8
GROUPS = 4            # head groups (tensor parallel)
HPG = H // GROUPS     # 4 heads per group
DH = D // H           # 64
GF = HPG * DH         # 256 features per group
DC = D // 128         # 8 contraction chunks of 128
QT = S // 512         # 4 tranches of 512
KT = S // 128         # 16 key tiles of 128 per section

BF16 = mybir.dt.bfloat16
F32 = mybir.dt.float32

LAG = 20   # trail (attn@v / normalize) lag behind the scores/exp stream
RLAG = 2   # rowsum chain-add lag behind the exp stream
NWARM = 16


def build_nc():
    nc = bacc.Bacc("TRN2", target_bir_lowering=False, debug=False,
                   num_devices=NCORES)

    # activations host-tiled [tranche, partition(=D-chunk row), dc, 512]
    kT = nc.dram_tensor("kT", [QT, 128, DC, 512], BF16, kind="ExternalInput").ap()
    qT = nc.dram_tensor("qT", [QT, 128, DC, 512], BF16, kind="ExternalInput").ap()
    vT = nc.dram_tensor("vT", [QT, 128, DC, 512], BF16, kind="ExternalInput").ap()
    wqT = nc.dram_tensor("wqT", [2, 128, DC, 128], BF16, kind="ExternalInput").ap()
    wkT = nc.dram_tensor("wkT", [2, 128, DC, 128], BF16, kind="ExternalInput").ap()
    wvT = nc.dram_tensor("wvT", [128, DC, GF], BF16, kind="ExternalInput").ap()
    woT = nc.dram_tensor("woT", [128, 2, D], BF16, kind="ExternalInput").ap()
    bq = nc.dram_tensor("bq", [GF], F32, kind="ExternalInput").ap()
    out = nc.dram_tensor("out_part", [S, D], BF16, kind="ExternalOutput").ap()

    with tile.TileContext(nc) as tc:
        with (
            tc.tile_pool(name="const", bufs=1) as const,
            tc.tile_pool(name="acts", bufs=1) as acts,
            tc.tile_pool(name="sc", bufs=2, space="PSUM") as scp,
            tc.tile_pool(name="av", bufs=2, space="PSUM") as avp,
            tc.tile_pool(name="misc", bufs=2, space="PSUM") as miscp,
            tc.tile_pool(name="exp", bufs=LAG + 2) as exps,
            tc.tile_pool(name="racc", bufs=3) as raccp,
            tc.tile_pool(name="bri", bufs=2) as brip,
            tc.tile_pool(name="ot", bufs=1) as otp,
            tc.tile_pool(name="small", bufs=2) as small,
            tc.tile_pool(name="outs", bufs=3) as outsp,
        ):
            # ---- constants (memsets run during the boilerplate head)
            ones128 = const.tile([128, 128], BF16, tag="ones128")
            nc.vector.memset(ones128, 1.0)
            ones64 = ones128[:, 0:64]
            warm_rhs = const.tile([128, 512], BF16, tag="warm_rhs")
            nc.vector.memset(warm_rhs, 0.0)

            # ---- input DMAs, in first-needed order; one dispatch each.
            # q/k weights are split by head-pair so the scores-critical head
            # only waits on 2.5 MB instead of 3.5 MB.
            wk_sb = [const.tile([128, DC, 128], BF16, tag=f"wk{p}", name=f"wk{p}")
                     for p in range(2)]
            wq_sb = [const.tile([128, DC, 128], BF16, tag=f"wq{p}", name=f"wq{p}")
                     for p in range(2)]
            k_sb = [acts.tile([128, DC, 512], BF16, tag=f"k{t}", name=f"k{t}")
                    for t in range(QT)]
            q_sb = [acts.tile([128, DC, 512], BF16, tag=f"q{t}", name=f"q{t}")
                    for t in range(QT)]
            v_sb = [acts.tile([128, DC, 512], BF16, tag=f"v{t}", name=f"v{t}")
                    for t in range(QT)]
            nc.sync.dma_start(out=wk_sb[0], in_=wkT[0, :, :, :])
            nc.sync.dma_start(out=k_sb[0], in_=kT[0, :, :, :])
            nc.sync.dma_start(out=wq_sb[0], in_=wqT[0, :, :, :])
            nc.sync.dma_start(out=q_sb[0], in_=qT[0, :, :, :])
            bq_sb = const.tile([128, 2], F32, tag="bq")
            nc.sync.dma_start(out=bq_sb, in_=bq.rearrange("(t p) -> p t", p=128))
            nc.sync.dma_start(out=k_sb[1], in_=kT[1, :, :, :])
            nc.sync.dma_start(out=k_sb[2], in_=kT[2, :, :, :])
            nc.sync.dma_start(out=k_sb[3], in_=kT[3, :, :, :])
            wv_sb = const.tile([128, DC, GF], BF16, tag="wv")
            nc.sync.dma_start(out=wv_sb, in_=wvT)
            nc.sync.dma_start(out=v_sb[0], in_=vT[0, :, :, :])
            nc.sync.dma_start(out=q_sb[1], in_=qT[1, :, :, :])
            nc.sync.dma_start(out=v_sb[1], in_=vT[1, :, :, :])
            nc.sync.dma_start(out=q_sb[2], in_=qT[2, :, :, :])
            nc.sync.dma_start(out=v_sb[2], in_=vT[2, :, :, :])
            nc.sync.dma_start(out=v_sb[3], in_=vT[3, :, :, :])
            nc.sync.dma_start(out=q_sb[3], in_=qT[3, :, :, :])
            nc.sync.dma_start(out=wk_sb[1], in_=wkT[1, :, :, :])
            nc.sync.dma_start(out=wq_sb[1], in_=wqT[1, :, :, :])
            wo_sb = const.tile([128, 2, D], BF16, tag="wo")
            nc.sync.dma_start(out=wo_sb, in_=woT)

            # ---- persistent projection outputs
            kp_sl = [[acts.tile([128, 512], BF16, tag=f"kp{p}_{t}",
                                name=f"kp{p}_{t}") for t in range(QT)]
                     for p in range(2)]
            qp_sl = [[acts.tile([128, 512], BF16, tag=f"qp{p}_{t}",
                                name=f"qp{p}_{t}") for t in range(QT)]
                     for p in range(2)]
            vp_t = [acts.tile([128, GF], BF16, tag=f"vp{st}", name=f"vp{st}")
                    for st in range(KT)]
            oT_tiles = [otp.tile([128, 2, 512], BF16, tag=f"ot{qt}",
                                 name=f"oT{qt}") for qt in range(QT)]

            # ---- PE warmup: spans the DMA head so HAM reaches 2.4 GHz
            # before the scores stream starts (dependency-free)
            warm_ps = miscp.tile([128, 512], F32, tag="misc", name="warm_ps")
            for _ in range(NWARM):
                nc.tensor.matmul(warm_ps, ones128, warm_rhs,
                                 start=True, stop=True)
            # WAR-ordered read so the psum slot releases right after warmup
            nc.vector.tensor_copy(warm_rhs, warm_ps)

            # ---- q/k projections, drip-fed in halves (4 dc chunks each).
            # A half-open group holds one misc ring slot; between its two
            # halves at most one other alloc may occur (safe: its last read
            # is always emitted before any later alloc's first write).
            _fs_state = {}

            def proj_fs_half(w_sb, b_sb, dst_sl, x_sb, pair, t, half):
                key = (id(dst_sl), pair, t)
                if half == 0:
                    _fs_state[key] = miscp.tile([128, 512], F32, tag="misc",
                                                name="ps_fs")
                ps = _fs_state[key]
                for dc in range(4 * half, 4 * half + 4):
                    nc.tensor.matmul(
                        ps,
                        w_sb[pair][:, dc, :],
                        x_sb[t][:, dc, :],
                        start=(dc == 0), stop=(dc == DC - 1),
                    )
                if half == 1:
                    del _fs_state[key]
                    if b_sb is None:
                        nc.vector.tensor_copy(dst_sl[pair][t], ps)
                    else:
                        nc.vector.tensor_scalar_add(
                            dst_sl[pair][t], ps, b_sb[:, pair:pair + 1])

            def proj_fs_group(w_sb, b_sb, dst_sl, x_sb, pair, t):
                proj_fs_half(w_sb, b_sb, dst_sl, x_sb, pair, t, 0)
                proj_fs_half(w_sb, b_sb, dst_sl, x_sb, pair, t, 1)

            def proj_fs_quarter(w_sb, b_sb, dst_sl, x_sb, pair, t, qq):
                key = (id(dst_sl), pair, t)
                if qq == 0:
                    _fs_state[key] = miscp.tile([128, 512], F32, tag="misc",
                                                name="ps_fs")
                ps = _fs_state[key]
                for dc in range(2 * qq, 2 * qq + 2):
                    nc.tensor.matmul(
                        ps,
                        w_sb[pair][:, dc, :],
                        x_sb[t][:, dc, :],
                        start=(dc == 0), stop=(dc == DC - 1),
                    )
                if qq == 3:
                    del _fs_state[key]
                    if b_sb is None:
                        nc.vector.tensor_copy(dst_sl[pair][t], ps)
                    else:
                        nc.vector.tensor_scalar_add(
                            dst_sl[pair][t], ps, b_sb[:, pair:pair + 1])

            def proj_v_group(st):
                ps = miscp.tile([128, GF], F32, tag="misc", name="ps_v")
                for dc in range(DC):
                    nc.tensor.matmul(
                        ps,
                        v_sb[st // 4][:, dc, 128 * (st % 4):128 * (st % 4) + 128],
                        wv_sb[:, dc, :],
                        start=(dc == 0), stop=(dc == DC - 1),
                    )
                nc.vector.tensor_copy(vp_t[st], ps)

            # prologue projections for stream position 0
            proj_fs_group(wk_sb, None, kp_sl, k_sb, 0, 0)
            proj_fs_group(wq_sb, bq_sb, qp_sl, q_sb, 0, 0)


            # drip-fed projection jobs at fixed stream positions.
            # needs: kp[0][t] by slot 4t; kp[1][t] by slot 16+4t;
            # qp[0][qt] by slot 32qt; qp[1][qt] by slot 32qt+16.
            scheduled = {}

            def sched(pos, job):
                scheduled.setdefault(pos, []).append(job)

            def sched_halves(pos, w, pair, t):
                wsb, bsb, dst, xs = ((wq_sb, bq_sb, qp_sl, q_sb) if w == "q"
                                     else (wk_sb, None, kp_sl, k_sb))
                for h in range(2):
                    sched(pos + h,
                          (lambda hh: lambda: proj_fs_half(
                              wsb, bsb, dst, xs, pair, t, hh))(h))

            def sched_quarters(pos, w, pair, t):
                wsb, bsb, dst, xs = ((wq_sb, bq_sb, qp_sl, q_sb) if w == "q"
                                     else (wk_sb, None, kp_sl, k_sb))
                for qq in range(4):
                    sched(pos + qq + (1 if pos + qq >= 67 and pos <= 67 else 0),
                          (lambda q2: lambda: proj_fs_quarter(
                              wsb, bsb, dst, xs, pair, t, q2))(qq))

            # v-projections dripped just ahead of their trail consumer
            # (kt consumed at slot LAG+kt)
            for _kt in range(8):
                sched(13 + _kt, (lambda k2: lambda: proj_v_group(k2))(_kt))
            for _kt in range(8, 16):
                sched(28 + (_kt - 8), (lambda k2: lambda: proj_v_group(k2))(_kt))

            sched_halves(2, "k", 0, 1)
            sched_halves(5, "k", 0, 2)
            sched_halves(9, "k", 0, 3)
            sched_halves(13, "q", 0, 1)
            sched_quarters(24, "q", 0, 2)
            sched_quarters(44, "q", 0, 3)
            sched_quarters(48, "q", 1, 0)
            sched_quarters(52, "k", 1, 0)
            sched_quarters(56, "k", 1, 1)
            sched_quarters(60, "k", 1, 2)
            sched_quarters(64, "k", 1, 3)
            sched_quarters(69, "q", 1, 1)
            sched_quarters(73, "q", 1, 2)
            sched_quarters(78, "q", 1, 3)

            # ---- output projection (per (qt, sj): 512 outputs in 2 halves)
            _op_state = {}

            def outproj_half(qt, sj, do):
                if do == 0:
                    _op_state[(qt, sj)] = outsp.tile([128, D], BF16, tag="os",
                                                     name="outt")
                outt = _op_state[(qt, sj)]
                oT_t = oT_tiles[qt]
                ps = miscp.tile([128, 512], F32, tag="misc", name="ps_o")
                for fc in range(2):
                    nc.tensor.matmul(
                        ps,
                        oT_t[:, fc, 128 * sj:128 * sj + 128],
                        wo_sb[:, fc, 512 * do:512 * do + 512],
                        start=(fc == 0), stop=(fc == 1),
                    )
                nc.vector.tensor_copy(outt[:, 512 * do:512 * do + 512], ps)
                if do == 1:
                    del _op_state[(qt, sj)]
                    row = 512 * qt + 128 * sj
                    nc.sync.dma_start(out=out[row:row + 128, :], in_=outt)

            # ---- per-section state
            # pair-major: sections 0-3 = head-pair 0 over qt 0-3, 4-7 = pair 1.
            # This defers the kp[1]/qp[1] projections to mid-stream slots,
            # easing the early-phase PE congestion.
            sections = [(qt, p) for p in range(2) for qt in range(QT)]
            stream = [(sec, kt) for sec in range(8) for kt in range(KT)]

            av_ps = {}           # sec -> av psum tile [128, 512]
            ex_store = {}        # stream idx -> exp tile
            racc_state = {}      # (sec, parity) -> (last_ex_or_None, acc_or_None)
            racc_final = {}      # sec -> accF tile
            jobs = []            # outproj job queue

            def radd(j):
                """DVE rowsum chain-add for the exp tile of stream slot j."""
                s, kt = stream[j]
                par = kt % 2
                ex = ex_store[j]
                key = (s, par)
                if kt < 2:
                    racc_state[key] = (ex, None)
                    return
                prev_ex, acc = racc_state[key]
                nacc = raccp.tile([128, 1024], BF16, tag="racc", name="racc")
                if acc is None:
                    nc.vector.tensor_add(nacc, prev_ex, ex)
                else:
                    nc.vector.tensor_add(nacc, acc, ex)
                racc_state[key] = (None, nacc)
                if kt == 15:
                    _, acc_e = racc_state.pop((s, 0))
                    _, acc_o = racc_state.pop((s, 1))
                    accf = raccp.tile([128, 1024], BF16, tag="racc",
                                      name="raccf")
                    nc.vector.tensor_add(accf, acc_e, acc_o)
                    racc_final[s] = accf

            def normalize(s):
                qt, p = sections[s]
                av = av_ps.pop(s)
                accf = racc_final.pop(s)
                # collapse rowsum partials across partitions; the [128, 64]
                # all-ones stationary replicates the per-query rowsum onto 64
                # partitions per head (col-tiled pair, concurrent)
                rbt = miscp.tile([128, 512], F32, tag="misc", name="rbt")
                nc.tensor.matmul(rbt[0:64, :], ones64, accf[:, 0:512],
                                 start=True, stop=True)
                nc.tensor.matmul(rbt[64:128, :], ones64, accf[:, 512:1024],
                                 start=True, stop=True)
                bri = brip.tile([128, 512], F32, tag="bri", name="bri")
                nc.vector.reciprocal_approx_fast(bri, rbt)
                nc.vector.tensor_mul(oT_tiles[qt][:, p, :], av, bri)
                if p == 1:
                    jobs.extend((qt, sj, do)
                                for sj in range(4) for do in range(2))

            def trail(jdx):
                s, kt = stream[jdx]
                qt, p = sections[s]
                if kt == 0:
                    av_ps[s] = avp.tile([128, 512], F32, tag="av", name="av")
                av = av_ps[s]
                exa = ex_store.pop(jdx)
                # col-tiled head pair: h(2p) on psum partitions 0-63
                # (array cols 0-63), h(2p+1) on 64-127 -- concurrent
                nc.tensor.matmul(
                    av[0:64, :],
                    vp_t[kt][:, 128 * p:128 * p + 64],
                    exa[:, 0:512],
                    start=(kt == 0), stop=(kt == KT - 1),
                )
                nc.tensor.matmul(
                    av[64:128, :],
                    vp_t[kt][:, 128 * p + 64:128 * p + 128],
                    exa[:, 512:1024],
                    start=(kt == 0), stop=(kt == KT - 1),
                )
                if kt == KT - 1:
                    normalize(s)

            # ---- the stream
            tcur = 0
            rcur = 0
            for i in range(len(stream) + LAG):
                if i < len(stream):
                    s, kt = stream[i]
                    qt, p = sections[s]
                    sc = scp.tile([128, 1024], F32, tag="sc", name="sc")
                    # row-packed head pair: 2p on array rows 0-63, 2p+1 on
                    # rows 64-127 (sequential; scores are psum-port-bound)
                    nc.tensor.matmul(
                        sc[:, 0:512],
                        kp_sl[p][kt // 4][0:64, 128 * (kt % 4):128 * (kt % 4) + 128],
                        qp_sl[p][qt][0:64, :],
                        start=True, stop=True,
                    )
                    nc.tensor.matmul(
                        sc[:, 512:1024],
                        kp_sl[p][kt // 4][64:128, 128 * (kt % 4):128 * (kt % 4) + 128],
                        qp_sl[p][qt][64:128, :],
                        start=True, stop=True,
                    )
                    ex = exps.tile([128, 1024], BF16, tag="exp", name="ex")
                    nc.scalar.activation(
                        out=ex, in_=sc,
                        func=mybir.ActivationFunctionType.Exp,
                        scale=1.0 / np.sqrt(DH),
                    )
                    ex_store[i] = ex
                    for fn in scheduled.pop(i, []):
                        fn()
                    if i not in scheduled and not _fs_state and jobs:
                        _q, _sj, _do = jobs.pop(0)
                        outproj_half(_q, _sj, _do)
                if i >= RLAG and rcur <= i - RLAG and rcur < len(stream):
                    radd(rcur)
                    rcur += 1
                if i >= LAG and tcur < len(stream):
                    trail(tcur)
                    tcur += 1
                # gently accelerate the trail (one extra AV every other
                # slot, fits the per-slot PE slack) so the tail is short
                if i >= 78 and i % 2 == 0 and tcur < len(stream) and tcur <= i - 4:
                    trail(tcur)
                    tcur += 1
            while rcur < len(stream):
                radd(rcur)
                rcur += 1
            while tcur < len(stream):
                trail(tcur)
                tcur += 1
            while jobs:
                _q, _sj, _do = jobs.pop(0)
                outproj_half(_q, _sj, _do)

    nc.compile()
    return nc


_CACHE = {}


def _get_nc():
    if "nc" not in _CACHE:
        _CACHE["nc"] = build_nc()
    return _CACHE["nc"]


def _tile_acts(x):
    # [S, D] -> transpose -> [(c p), (t s)] -> [t, p, c, s] contiguous
    bf16 = ml_dtypes.bfloat16
    xt = x.T.reshape(DC, 128, QT, 512).transpose(2, 1, 0, 3)
    return np.ascontiguousarray(xt).astype(bf16)


def _tile_w(w):
    # [D, GF] -> [(c p), f] -> [p, c, f] contiguous
    bf16 = ml_dtypes.bfloat16
    return np.ascontiguousarray(
        w.reshape(DC, 128, GF).transpose(1, 0, 2)).astype(bf16)


def _tile_w_pair(w):
    # [D, GF] -> [pair, p, c, 128] contiguous
    bf16 = ml_dtypes.bfloat16
    return np.ascontiguousarray(
        w.reshape(DC, 128, 2, 128).transpose(2, 1, 0, 3)).astype(bf16)


def _prep_inputs(query, key, value, in_proj_w, in_proj_b, out_proj_w):
    bf16 = ml_dtypes.bfloat16
    wq, wk, wv = (in_proj_w[0:D], in_proj_w[D:2 * D], in_proj_w[2 * D:3 * D])
    bq = in_proj_b[0:D]

    qT = [_tile_acts(query[b]) for b in range(B)]
    kT = [_tile_acts(key[b]) for b in range(B)]
    vT = [_tile_acts(value[b]) for b in range(B)]

    in_maps = []
    for b in range(B):
        for g in range(GROUPS):
            fs = slice(GF * g, GF * (g + 1))
            woT = out_proj_w[:, fs].T   # [GF, D]
            in_maps.append({
                "qT": qT[b], "kT": kT[b], "vT": vT[b],
                "wqT": _tile_w_pair(np.ascontiguousarray(wq[fs].T)),
                "wkT": _tile_w_pair(np.ascontiguousarray(wk[fs].T)),
                "wvT": _tile_w(np.ascontiguousarray(wv[fs].T)),
                "woT": np.ascontiguousarray(
                    woT.reshape(2, 128, D).transpose(1, 0, 2)).astype(bf16),
                "bq": np.ascontiguousarray(bq[fs]).astype(np.float32),
            })
    return in_maps


def kernel(query, key, value, in_proj_w, in_proj_b, out_proj_w, out_proj_b,
           mask_w, mask_b, _run_kwargs=None):
    query = np.asarray(query, np.float32)
    key = np.asarray(key, np.float32)
    value = np.asarray(value, np.float32)
    in_proj_w = np.asarray(in_proj_w, np.float32)
    in_proj_b = np.asarray(in_proj_b, np.float32)
    out_proj_w = np.asarray(out_proj_w, np.float32)
    out_proj_b = np.asarray(out_proj_b, np.float32)
    mask_w = np.asarray(mask_w, np.float32)
    mask_b = np.asarray(mask_b, np.float32)

    in_maps = _prep_inputs(query, key, value, in_proj_w, in_proj_b, out_proj_w)
    nc = _get_nc()
    for _attempt in range(3):
        res = run_bass_kernel_spmd(nc, in_maps, core_ids=list(range(NCORES)),
                                   **(_run_kwargs or {}))
        parts = [np.asarray(r["out_part"], np.float32) for r in res.results]
        # guard against rare transient device glitches: partial outputs are
        # normally bounded well under 1
        if all(np.isfinite(p).all() and np.abs(p).max() < 100.0 for p in parts):
            break
    # the v-projection bias is exact to fold into the output bias:
    # o = attn @ (v + bv) -> attn@v + bv, so out += bv @ Wo^T
    bv = in_proj_b[2 * D:3 * D]
    eff_bias = out_proj_b + bv @ out_proj_w.T
    mha = np.stack(
        [sum(parts[b * GROUPS + g] for g in range(GROUPS)) for b in range(B)],
        axis=0,
    ) + eff_bias[None, None, :].astype(np.float32)

    logit = (query[:, -1] @ mask_w.T + mask_b).astype(np.float64)
    m = (1.0 / (1.0 + np.exp(-logit))).astype(np.float32).reshape(B)
    c = np.where(m < 0.1, np.float32(1.0) - m, np.float32(1.0))

    out_full = c[:, None, None, None] * mha[None, :, :, :]
    if _run_kwargs is not None:
        _CACHE["last_results"] = res
    return out_full.astype(np.float32)
